# revision 1
# baseline (speedup 1.0000x reference)
"""Trainium2 Bass kernel for nn_Attention_10582799417937.

Data-parallel over batch (32 -> 4 per core x 8 cores), weights replicated.
Per-core pipeline (per batch):
  depthwise 3x3 convs (q-path on DVE/ACT, kv-path on gpsimd)
  -> pointwise projections (PE matmuls, bf16)
  -> attention computed transposed: dotsT[j,i] = k_h^T . q_h^T so the
     attention matrix never needs transposing; softmax denominators via
     ones-mask matmuls on PE (replicated across partitions), normalize on DVE.
  -> output projection (PE) -> DRAM.
All BN affine and the V-path bias are folded into weights on the host.
"""
import sys
import numpy as np
import ml_dtypes

sys.path.insert(0, "/opt/trn_rl_repo")

import concourse.bass as bass
import concourse.mybir as mybir
import concourse.tile as tile
from concourse import bacc
from concourse.bass_utils import run_bass_kernel_spmd

# ---- problem constants (hardcoded per spec) ----
B, C, H, W = 32, 384, 32, 32
HEADS, D = 6, 64
INNER = HEADS * D          # 384
SCALE = D ** -0.5
EPS = 1e-5
N_CORES = 8
B_LOC = B // N_CORES       # 4
HW = H * W                 # 1024
HK, WK = H // 2, W // 2
JK = HK * WK               # 256
KC = C // 128              # 3 channel chunks
MC = INNER // 128          # 3 inner chunks (also head pairs)
NPAIR = HEADS // 2         # 3

BF16 = mybir.dt.bfloat16
F32 = mybir.dt.float32
AL = mybir.AluOpType
AF = mybir.ActivationFunctionType

# padded layout: [128, 34 rows x 34 cols]; data at rows 1..32, cols 1..32.
# Two batches are packed per xp tile ([128, 2, 34, 34]).
PADR = 34
PADN = PADR * PADR

CONV2_ON_PE = True
ACT_MUL_TAPS = {(0, 0), (0, 2)}

# tap order: center tap first (initializes accumulator)
ORDER = [(1, 1), (1, 0), (1, 2), (0, 0), (0, 1), (0, 2), (2, 0), (2, 1), (2, 2)]


def _conv_s1_dve(nc, scratch, xpv, taps_sb, acc):
    """Stride-1 3x3 depthwise conv over a 2-batch packed tile (v4).
    acc: [128, 2048] bf16."""
    accv = acc.rearrange("p (b hw) -> p b hw", b=2)
    for idx, (dy, dx) in enumerate(ORDER):
        t = dy * 3 + dx
        in_view = xpv[:, :, dy:dy + 32, dx:dx + W]
        scal = taps_sb[:, t:t + 1]
        if idx == 0:
            nc.vector.tensor_scalar_mul(accv, in_view, scal)
            continue
        tmp = scratch.tile([128, 2 * HW], BF16, tag="cscr", name="cscr")
        tv = tmp[:].rearrange("p (b hw) -> p b hw", b=2)
        if (dy, dx) in ACT_MUL_TAPS:
            nc.scalar.mul(tv, in_view, scal)
        else:
            nc.vector.tensor_scalar_mul(tv, in_view, scal)
        nc.vector.tensor_tensor(acc, acc, tmp[:], AL.add)


def _conv_s2_dve(nc, scratch, xpv, taps_sb, acc):
    """Stride-2 3x3 depthwise conv, 2-batch packed (v4). DVE multiplies
    feed a serial gpsimd accumulate chain. acc: [128, 512] bf16."""
    accv = acc.rearrange("p (b hw) -> p b hw", b=2)
    for idx, (dy, dx) in enumerate(ORDER):
        t = dy * 3 + dx
        in_view = xpv[:, :, dy:dy + 31:2, dx:dx + 31:2]
        scal = taps_sb[:, t:t + 1]
        if idx == 0:
            nc.vector.tensor_scalar_mul(accv, in_view, scal)
            continue
        tmp = scratch.tile([128, 2 * JK], BF16, tag="kscr", name="kscr")
        tv = tmp[:].rearrange("p (b hw) -> p b hw", b=2)
        nc.vector.tensor_scalar_mul(tv, in_view, scal)
        nc.gpsimd.tensor_tensor(acc, acc, tmp[:], AL.add)


def _conv_s1_pe(nc, psc, qdiag_sb, xpv, h, kc_, y1t):
    """Stride-1 conv for one batch entirely on PE: 9 accumulating
    diag(w_t) @ x_window matmuls per 512-half, ACT evict -> y1t bf16."""
    for n2 in range(2):
        pst = psc.tile([128, 512], F32, tag="psc", name="pst")
        for ti, (dy, dx) in enumerate(ORDER):
            dg = qdiag_sb[ti * KC + kc_]
            rhs = xpv[:, h, dy + n2 * 16:dy + n2 * 16 + 16, dx:dx + W]
            nc.tensor.matmul(pst[:], dg, rhs,
                             start=(ti == 0), stop=(ti == len(ORDER) - 1))
        nc.scalar.activation(y1t[:, n2 * 512:(n2 + 1) * 512], pst[:], AF.Copy)


def _conv_s2_pe(nc, psc, kvdiag_sb, xpv, h, kc_, y2t):
    """Stride-2 conv for one batch on PE: 9 accumulating matmuls with
    stride-2 windows, ACT evict -> y2t [128, 256] bf16."""
    pst = psc.tile([128, JK], F32, tag="psc", name="pstk")
    for ti, (dy, dx) in enumerate(ORDER):
        dg = kvdiag_sb[ti * KC + kc_]
        rhs = xpv[:, h, dy:dy + 31:2, dx:dx + 31:2]
        nc.tensor.matmul(pst[:], dg, rhs,
                         start=(ti == 0), stop=(ti == len(ORDER) - 1))
    nc.scalar.activation(y2t, pst[:], AF.Copy)


def build_nc():
    nc = bacc.Bacc(None, target_bir_lowering=False)
    x_ext = nc.declare_dram_parameter("x", [B_LOC, C, H, W], BF16, False)
    aq_ext = nc.declare_dram_parameter("aq", [C, INNER], BF16, False)
    ak_ext = nc.declare_dram_parameter("ak", [C, INNER], BF16, False)
    av_ext = nc.declare_dram_parameter("av", [C, INNER], BF16, False)
    w2_ext = nc.declare_dram_parameter("w2", [INNER, C], BF16, False)
    qt_ext = nc.declare_dram_parameter("qtap", [C, 9], F32, False)
    kt_ext = nc.declare_dram_parameter("kvtap", [C, 9], F32, False)
    qd_ext = nc.declare_dram_parameter("qdiag", [9 * C, 128], BF16, False)
    kd_ext = nc.declare_dram_parameter("kvdiag", [9 * C, 128], BF16, False)
    bq_ext = nc.declare_dram_parameter("bq", [INNER, 1], F32, False)
    bk_ext = nc.declare_dram_parameter("bk", [INNER, 1], F32, False)
    b2_ext = nc.declare_dram_parameter("b2", [C, 1], F32, False)
    out_ext = nc.declare_dram_parameter("out", [B_LOC, C, H, W], F32, True)

    from contextlib import ExitStack
    with tile.TileContext(nc) as tc, ExitStack() as ctx:
        wpool = ctx.enter_context(tc.tile_pool(name="weights", bufs=1))
        xstage = ctx.enter_context(tc.tile_pool(name="xs", bufs=4))
        xpool = ctx.enter_context(tc.tile_pool(name="xp", bufs=5))
        scratch = ctx.enter_context(tc.tile_pool(name="scratch", bufs=3))
        y1pool = ctx.enter_context(tc.tile_pool(name="y1", bufs=7))
        y2pool = ctx.enter_context(tc.tile_pool(name="y2", bufs=7))
        qpool = ctx.enter_context(tc.tile_pool(name="q", bufs=6))
        kpool = ctx.enter_context(tc.tile_pool(name="k", bufs=6))
        vpool = ctx.enter_context(tc.tile_pool(name="v", bufs=4))
        epool = ctx.enter_context(tc.tile_pool(name="et", bufs=8))
        rpool = ctx.enter_context(tc.tile_pool(name="recip", bufs=3))
        opool = ctx.enter_context(tc.tile_pool(name="outT", bufs=6))
        fpool = ctx.enter_context(tc.tile_pool(name="fin", bufs=3))
        ps2 = ctx.enter_context(tc.tile_pool(name="ps2", bufs=3, space="PSUM"))
        psc = ctx.enter_context(tc.tile_pool(name="psc", bufs=2, space="PSUM"))

        # ---- load weights (persistent) ----
        def wload(ext, kc_, shape, dtype, tag):
            t = wpool.tile(shape, dtype, tag=f"{tag}{kc_}", name=f"{tag}{kc_}")
            nc.sync.dma_start(t[:], ext[kc_ * 128:(kc_ + 1) * 128, :])
            return t

        aq_sb = [wload(aq_ext, i, [128, INNER], BF16, "aq") for i in range(KC)]
        ak_sb = [wload(ak_ext, i, [128, INNER], BF16, "ak") for i in range(KC)]
        av_sb = [wload(av_ext, i, [128, INNER], BF16, "av") for i in range(KC)]
        w2_sb = [wload(w2_ext, i, [128, C], BF16, "w2") for i in range(MC)]
        qt_sb = [wload(qt_ext, i, [128, 9], F32, "qt") for i in range(KC)]
        kt_sb = [wload(kt_ext, i, [128, 9], F32, "kt") for i in range(KC)]
        bq_sb = [wload(bq_ext, i, [128, 1], F32, "bq") for i in range(MC)]
        bk_sb = [wload(bk_ext, i, [128, 1], F32, "bk") for i in range(MC)]
        b2_sb = [wload(b2_ext, i, [128, 1], F32, "b2") for i in range(MC)]
        # all 27 diag blocks per path in one wide tile (single fast DMA)
        def dgload(ext, tag):
            t = wpool.tile([128, 9 * KC * 128], BF16, tag=tag, name=tag)
            nc.sync.dma_start(
                t[:].rearrange("p (blk d) -> p blk d", d=128),
                ext[:, :].rearrange("(blk p) d -> p blk d", p=128))
            return [t[:, i * 128:(i + 1) * 128] for i in range(9 * KC)]

        qdiag_sb = dgload(qd_ext, "qdall")
        kvdiag_sb = dgload(kd_ext, "kdall")

        # ones-masks for denominator matmuls
        maskA = wpool.tile([128, 128], BF16, tag="maskA", name="maskA")
        maskB = wpool.tile([128, 128], BF16, tag="maskB", name="maskB")
        nc.gpsimd.memset(maskA[:], 0.0)
        nc.gpsimd.memset(maskA[:, 0:64], 1.0)
        nc.gpsimd.memset(maskB[:], 0.0)
        nc.gpsimd.memset(maskB[:, 64:128], 1.0)

        def conv_pair(b01, on_pe):
            """DMA x for batches (2*b01, 2*b01+1) + both depthwise convs.
            on_pe selects the PE (diag matmul) or DVE implementation.
            Returns (y1 per-batch lists, y2 per-batch lists) of APs."""
            b = 2 * b01
            y1 = [[], []]
            y2 = [[], []]
            for kc_ in range(KC):
                xs = xstage.tile([128, 2 * HW], BF16, tag="xs", name="xs")
                src = x_ext[b:b + 2, kc_ * 128:(kc_ + 1) * 128, :, :]
                nc.scalar.dma_start(
                    xs[:].rearrange("p (b hw) -> p b hw", b=2),
                    src.rearrange("b c h w -> c b (h w)"))
                xp = xpool.tile([128, 2 * PADN], BF16, tag="xp", name="xp")
                xpv = xp[:].rearrange("p (b r c) -> p b r c", b=2, c=PADR)
                nc.gpsimd.memset(xpv[:, :, 0:1, :], 0.0)
                nc.gpsimd.memset(xpv[:, :, 33:34, :], 0.0)
                nc.gpsimd.memset(xpv[:, :, 1:33, 0:1], 0.0)
                nc.gpsimd.memset(xpv[:, :, 1:33, 33:34], 0.0)
                copy_eng = nc.scalar if on_pe else nc.vector
                if on_pe:
                    nc.scalar.copy(
                        xpv[:, :, 1:33, 1:33],
                        xs[:].rearrange("p (b h w) -> p b h w", b=2, w=W))
                else:
                    nc.vector.tensor_copy(
                        xpv[:, :, 1:33, 1:33],
                        xs[:].rearrange("p (b h w) -> p b h w", b=2, w=W))
                if on_pe:
                    for h in range(2):
                        a1 = y1pool.tile([128, HW], BF16, tag="y1", name="y1", bufs=6)
                        _conv_s1_pe(nc, psc, qdiag_sb, xpv, h, kc_, a1[:])
                        y1[h].append(a1[:])
                        a2 = y2pool.tile([128, JK], BF16, tag="y2", name="y2", bufs=6)
                        _conv_s2_pe(nc, psc, kvdiag_sb, xpv, h, kc_, a2[:])
                        y2[h].append(a2[:])
                else:
                    a1 = y1pool.tile([128, 2 * HW], BF16, tag="y1p", name="y1p", bufs=4)
                    _conv_s1_dve(nc, scratch, xpv, qt_sb[kc_], a1[:])
                    a2 = y2pool.tile([128, 2 * JK], BF16, tag="y2p", name="y2p", bufs=4)
                    _conv_s2_dve(nc, scratch, xpv, kt_sb[kc_], a2[:])
                    for h in range(2):
                        y1[h].append(a1[:, h * HW:(h + 1) * HW])
                        y2[h].append(a2[:, h * JK:(h + 1) * JK])
            return y1, y2

        def ab_phase(b, y1, y2):
            ev_dve = b >= 2
            # ---- stage A: q = Aq^T.T @ y1 + bq ----
            q_sb = []
            for mc_ in range(MC):
                qt = qpool.tile([128, HW], BF16, tag="q", name="qsb")
                ps = ps2.tile([128, 1024], F32, tag="ps2", name="psA")
                for n2 in range(2):
                    for kc_ in range(KC):
                        nc.tensor.matmul(
                            ps[:, n2 * 512:(n2 + 1) * 512],
                            aq_sb[kc_][:, mc_ * 128:(mc_ + 1) * 128],
                            y1[kc_][:, n2 * 512:(n2 + 1) * 512],
                            start=(kc_ == 0), stop=(kc_ == KC - 1))
                if ev_dve:
                    nc.vector.tensor_scalar_add(qt[:], ps[:], bq_sb[mc_][:])
                else:
                    nc.scalar.activation(qt[:], ps[:], AF.Identity,
                                         bias=bq_sb[mc_][:], scale=1.0)
                q_sb.append(qt)

            # ---- stage Bk ----
            k_sb = []
            for mc_ in range(MC):
                kt = kpool.tile([128, JK], BF16, tag="k", name="ksb")
                ps = psc.tile([128, JK], F32, tag="psc", name="psBk")
                for kc_ in range(KC):
                    nc.tensor.matmul(
                        ps[:], ak_sb[kc_][:, mc_ * 128:(mc_ + 1) * 128], y2[kc_],
                        start=(kc_ == 0), stop=(kc_ == KC - 1))
                nc.scalar.activation(kt[:], ps[:], AF.Identity,
                                     bias=bk_sb[mc_][:], scale=1.0)
                k_sb.append(kt)

            # ---- stage Bv: vT[j, hd] ----
            vT_sb = []
            for jc in range(2):
                vt = vpool.tile([128, INNER], BF16, tag="v", name="vsb")
                ps = psc.tile([128, INNER], F32, tag="psc", name="psBv")
                for kc_ in range(KC):
                    nc.tensor.matmul(
                        ps[:], y2[kc_][:, jc * 128:(jc + 1) * 128], av_sb[kc_][:],
                        start=(kc_ == 0), stop=(kc_ == KC - 1))
                nc.scalar.activation(vt[:], ps[:], AF.Copy)
                vT_sb.append(vt)
            return q_sb, k_sb, vT_sb

        def attn_pair(b, q_sb, k_sb, vT_sb, p):
            et = [[epool.tile([128, HW], BF16, tag="et", name="et")
                   for _ in range(2)] for _ in range(2)]
            for h01 in range(2):
                hs = h01 * 64
                for jc in range(2):
                    psd = ps2.tile([128, 1024], F32, tag="ps2", name="psd")
                    for ic in range(2):
                        nc.tensor.matmul(
                            psd[:, ic * 512:(ic + 1) * 512],
                            k_sb[p][hs:hs + 64, jc * 128:(jc + 1) * 128],
                            q_sb[p][hs:hs + 64, ic * 512:(ic + 1) * 512],
                            start=True, stop=True,
                            tile_position=(hs, 0))
                    nc.scalar.activation(et[h01][jc][:], psd[:],
                                         AF.Exp, scale=SCALE)

            rec = rpool.tile([128, HW], F32, tag="recip", name="recip")
            psn = ps2.tile([128, 1024], F32, tag="ps2", name="psn")
            mms = [(maskA, et[0][0]), (maskA, et[0][1]),
                   (maskB, et[1][0]), (maskB, et[1][1])]
            for ic in range(2):
                for mi, (msk, e) in enumerate(mms):
                    nc.tensor.matmul(
                        psn[:, ic * 512:(ic + 1) * 512], msk[:],
                        e[:, ic * 512:(ic + 1) * 512],
                        start=(mi == 0), stop=(mi == len(mms) - 1))
            nc.vector.reciprocal_approx_fast(out=rec[:], in_=psn[:])

            ot = opool.tile([128, HW], BF16, tag="outT", name="outT")
            pso = ps2.tile([128, 1024], F32, tag="ps2", name="pso")
            for ic in range(2):
                for h01 in range(2):
                    hs = h01 * 64
                    for jc in range(2):
                        nc.tensor.matmul(
                            pso[hs:hs + 64, ic * 512:(ic + 1) * 512],
                            vT_sb[jc][:, p * 128 + hs:p * 128 + hs + 64],
                            et[h01][jc][:, ic * 512:(ic + 1) * 512],
                            start=(jc == 0), stop=(jc == 1),
                            tile_position=(0, hs))
            nc.vector.tensor_tensor(ot[:], pso[:], rec[:], AL.mult)
            return ot

        def e_phase(b, outT_sb):
            ev_dve = b >= 2
            for mc_ in range(MC):
                fin = fpool.tile([128, HW], F32, tag="fin", name="fin")
                ps = ps2.tile([128, 1024], F32, tag="ps2", name="psE")
                for n2 in range(2):
                    for p in range(NPAIR):
                        nc.tensor.matmul(
                            ps[:, n2 * 512:(n2 + 1) * 512],
                            w2_sb[p][:, mc_ * 128:(mc_ + 1) * 128],
                            outT_sb[p][:, n2 * 512:(n2 + 1) * 512],
                            start=(p == 0), stop=(p == NPAIR - 1))
                if ev_dve:
                    nc.vector.tensor_scalar_add(fin[:], ps[:], b2_sb[mc_][:])
                else:
                    nc.scalar.activation(fin[:], ps[:], AF.Identity,
                                         bias=b2_sb[mc_][:], scale=1.0)
                nc.sync.dma_start(
                    out_ext[b, mc_ * 128:(mc_ + 1) * 128, :, :],
                    fin[:].rearrange("p (h w) -> p h w", w=W))

        def rest_phase(b, y1, y2):
            q_sb, k_sb, vT_sb = ab_phase(b, y1, y2)
            outT_sb = [attn_pair(b, q_sb, k_sb, vT_sb, p) for p in range(NPAIR)]
            e_phase(b, outT_sb)

        # conv runs a pair ahead of the matmul phases (v4 schedule);
        # CONV2_ON_PE routes the second pair's conv to the PE path
        y1a, y2a = conv_pair(0, on_pe=False)
        y1b, y2b = conv_pair(1, on_pe=CONV2_ON_PE)
        rest_phase(0, y1a[0], y2a[0])
        rest_phase(1, y1a[1], y2a[1])
        # stage-interleave the tail batches so their attention chains fill
        # each other's stalls in the in-order PE stream
        s2 = ab_phase(2, y1b[0], y2b[0])
        s3 = ab_phase(3, y1b[1], y2b[1])
        o2, o3 = [], []
        for p in range(NPAIR):
            o2.append(attn_pair(2, *s2, p))
            o3.append(attn_pair(3, *s3, p))
        e_phase(2, o2)
        e_phase(3, o3)

    nc.compile()
    return nc


_NC_CACHE = None


def _get_nc():
    global _NC_CACHE
    if _NC_CACHE is None:
        _NC_CACHE = build_nc()
    return _NC_CACHE


def _prep_host(inputs):
    """Fold BN into pointwise weights; fold v-bias into final bias."""
    f32 = np.float32
    bf16 = ml_dtypes.bfloat16
    inv_q = (inputs['q_gamma'] / np.sqrt(inputs['q_var'] + EPS)).astype(f32)
    sh_q = (inputs['q_beta'] - inputs['q_mean'] * inv_q).astype(f32)
    A_q = (inputs['q_pw'] * inv_q[None, :]).astype(f32)
    b_q = (inputs['q_pw'].astype(f32) @ sh_q).astype(f32)

    inv_kv = (inputs['kv_gamma'] / np.sqrt(inputs['kv_var'] + EPS)).astype(f32)
    sh_kv = (inputs['kv_beta'] - inputs['kv_mean'] * inv_kv).astype(f32)
    A_kv = (inputs['kv_pw'] * inv_kv[None, :]).astype(f32)
    b_kv = (inputs['kv_pw'].astype(f32) @ sh_kv).astype(f32)
    A_k, A_v = A_kv[:INNER], A_kv[INNER:]
    b_k, b_v = b_kv[:INNER], b_kv[INNER:]

    W2 = inputs['out_w'].astype(f32)
    b2 = (inputs['out_b'].astype(f32) + W2 @ b_v).astype(f32)

    def diag_blocks(taps):
        out = np.zeros((9 * C, 128), f32)
        for ti, (dy, dx) in enumerate(ORDER):
            t = dy * 3 + dx
            for kc_ in range(KC):
                blk = np.diag(taps[kc_ * 128:(kc_ + 1) * 128, t])
                out[ti * C + kc_ * 128:ti * C + (kc_ + 1) * 128, :] = blk
        return out

    qdiag = diag_blocks(inputs['q_dw'].reshape(C, 9).astype(f32))
    kvdiag = diag_blocks(inputs['kv_dw'].reshape(C, 9).astype(f32))

    return {
        'qdiag': qdiag.astype(bf16),
        'kvdiag': kvdiag.astype(bf16),
        'aq': np.ascontiguousarray(A_q.T).astype(bf16),
        'ak': np.ascontiguousarray(A_k.T).astype(bf16),
        'av': np.ascontiguousarray(A_v.T).astype(bf16),
        'w2': np.ascontiguousarray(W2.T).astype(bf16),
        'qtap': np.ascontiguousarray(inputs['q_dw'].reshape(C, 9)).astype(f32),
        'kvtap': np.ascontiguousarray(inputs['kv_dw'].reshape(C, 9)).astype(f32),
        'bq': b_q.reshape(INNER, 1),
        'bk': b_k.reshape(INNER, 1),
        'b2': b2.reshape(C, 1),
    }


def kernel(**inputs):
    inputs = {k: np.asarray(v) for k, v in inputs.items()}
    nc = _get_nc()
    wmap = _prep_host(inputs)
    xb = inputs['x'].astype(ml_dtypes.bfloat16)
    in_maps = []
    for c in range(N_CORES):
        m = dict(wmap)
        m['x'] = np.ascontiguousarray(xb[c * B_LOC:(c + 1) * B_LOC])
        in_maps.append(m)
    res = run_bass_kernel_spmd(nc, in_maps, core_ids=list(range(N_CORES)))
    shards = [res.results[i]['out'] for i in range(N_CORES)]
    return np.concatenate(shards, axis=0).astype(np.float32)



# revision 14
# speedup vs baseline: 1.0079x; 1.0079x over previous
"""Trainium2 Bass kernel for nn_Attention_10582799417937 (v5: fp8 + DoubleRow).

Data-parallel over batch (32 -> 4 per core x 8 cores), weights replicated.
All matmuls except the final output projection run in fp8(e4m3); wherever the
contraction spans two 128-deep K-tiles they are merged into a single
MatmulPerfMode.DoubleRow instruction (2 k-subtiles per pass):
  - depthwise 3x3 convs on PE as diag matmuls, taps paired 2-per-instruction
    via overlapping-window APs (5 DR matmuls instead of 9 plain)
  - q/k/v pointwise projections: C=384 contraction = 1 DR (256) + 1 plain (128)
  - softmax denominators via ones-mask matmuls, jc-paired DR
  - attn@v, jc-paired DR
Final projection stays bf16 (fp8 there costs ~3% rel err: attention is a
near-uniform average here so output variation is small vs its mean).
Elementwise work is spread: exp on ACT, big psum evictions + recip/normalize
on DVE, small evictions + normalize(odd b) on GPSIMD.
"""
import sys
import numpy as np
import ml_dtypes

sys.path.insert(0, "/opt/trn_rl_repo")

import concourse.bass as bass
import concourse.mybir as mybir
import concourse.tile as tile
from concourse import bacc
from concourse.bass_utils import run_bass_kernel_spmd

# ---- problem constants (hardcoded per spec) ----
B, C, H, W = 32, 384, 32, 32
HEADS, D = 6, 64
INNER = HEADS * D          # 384
SCALE = D ** -0.5
EPS = 1e-5
N_CORES = 8
B_LOC = B // N_CORES       # 4
HW = H * W                 # 1024
HK, WK = H // 2, W // 2
JK = HK * WK               # 256
KC = C // 128              # 3 channel chunks
MC = INNER // 128          # 3 inner chunks (also head pairs)
NPAIR = HEADS // 2         # 3

BF16 = mybir.dt.bfloat16
F32 = mybir.dt.float32
FP8 = mybir.dt.float8e4
NP_FP8 = ml_dtypes.float8_e4m3
AL = mybir.AluOpType
AF = mybir.ActivationFunctionType
DR = mybir.MatmulPerfMode.DoubleRow

WS = 16.0                  # fp8 weight prescale (folded out at psum evict)
IWS = 1.0 / WS

# padded per-batch image: [34 rows x 34 cols]; data at rows 1..32, cols 1..32.
PADR = 34
PADN = PADR * PADR

# tap pairs for DoubleRow conv: (tapA, tapB) by (dy, dx); last pair reuses the
# (2,2) window with a zero diag block as subtile B (delta 0).
TAP_PAIRS = [((0, 0), (0, 1)), ((0, 2), (1, 0)), ((1, 1), (1, 2)),
             ((2, 0), (2, 1)), ((2, 2), None)]
N_TAPS = 2 * len(TAP_PAIRS)  # 10 diag blocks (incl. zero pad)


def _off(t):
    return t[0] * PADR + t[1]


def _pair_win(xp4, bb, tA, tB, n2=None, stride=1):
    """Overlapping-window rhs AP [128, 2, rows, cols] for a DoubleRow conv
    matmul: subtile dim selects tap A/B via a hand-set stride."""
    dy, dx = tA
    if stride == 1:
        w = xp4[:, bb, dy + 16 * n2:dy + 16 * n2 + 16, dx:dx + W]
    else:
        w = xp4[:, bb, dy:dy + 31:2, dx:dx + 31:2]
    u = w.unsqueeze(1)
    delta = (_off(tB) - _off(tA)) if tB is not None else 0
    u.ap[1] = [delta, 2]
    return u


def build_nc():
    nc = bacc.Bacc(None, target_bir_lowering=False)
    x_ext = nc.declare_dram_parameter("x", [B_LOC, C, H, W], FP8, False)
    aq_ext = nc.declare_dram_parameter("aq", [C, INNER], FP8, False)
    ak_ext = nc.declare_dram_parameter("ak", [C, INNER], FP8, False)
    av_ext = nc.declare_dram_parameter("av", [C, INNER], FP8, False)
    w2_ext = nc.declare_dram_parameter("w2", [INNER, C], BF16, False)
    qd_ext = nc.declare_dram_parameter("qdiag", [N_TAPS * C, 128], FP8, False)
    kd_ext = nc.declare_dram_parameter("kvdiag", [N_TAPS * C, 128], FP8, False)
    bq_ext = nc.declare_dram_parameter("bq", [INNER, 1], F32, False)
    bk_ext = nc.declare_dram_parameter("bk", [INNER, 1], F32, False)
    b2_ext = nc.declare_dram_parameter("b2", [C, 1], F32, False)
    out_ext = nc.declare_dram_parameter("out", [B_LOC, C, H, W], F32, True)

    from contextlib import ExitStack
    with tile.TileContext(nc) as tc, ExitStack() as ctx:
        wpool = ctx.enter_context(tc.tile_pool(name="weights", bufs=1))
        xpool = ctx.enter_context(tc.tile_pool(name="xp", bufs=6))
        y1pool = ctx.enter_context(tc.tile_pool(name="y1", bufs=4))
        y2pool = ctx.enter_context(tc.tile_pool(name="y2", bufs=4))
        qpool = ctx.enter_context(tc.tile_pool(name="q", bufs=6))
        kpool = ctx.enter_context(tc.tile_pool(name="k", bufs=6))
        vpool = ctx.enter_context(tc.tile_pool(name="v", bufs=2))
        epool = ctx.enter_context(tc.tile_pool(name="et", bufs=12))
        rpool = ctx.enter_context(tc.tile_pool(name="recip", bufs=3))
        opool = ctx.enter_context(tc.tile_pool(name="outT", bufs=6))
        fpool = ctx.enter_context(tc.tile_pool(name="fin", bufs=3))
        psbig = ctx.enter_context(tc.tile_pool(name="psbig", bufs=3, space="PSUM"))
        pssml = ctx.enter_context(tc.tile_pool(name="pssml", bufs=2, space="PSUM"))

        # ---- load weights (persistent) ----
        def wload(ext, shape, dtype, tag):
            t = wpool.tile(shape, dtype, tag=tag, name=tag)
            nc.sync.dma_start(t[:], ext[:, :])
            return t

        # full [384, INNER] as 3 kc chunks into one [128, 3*INNER] tile;
        # kc0,kc1 are adjacent so they form the DoubleRow subtile pair
        def wload3(ext, tag):
            t = wpool.tile([128, KC * INNER], FP8, tag=tag, name=tag)
            nc.sync.dma_start(
                t[:].rearrange("p (kc m) -> p kc m", kc=KC),
                ext[:, :].rearrange("(kc p) m -> p kc m", p=128))
            return t

        aq_sb = wload3(aq_ext, "aq")
        ak_sb = wload3(ak_ext, "ak")
        av_sb = wload3(av_ext, "av")
        aqv = aq_sb[:].rearrange("p (kc m) -> p kc m", kc=KC)
        akv = ak_sb[:].rearrange("p (kc m) -> p kc m", kc=KC)
        avv = av_sb[:].rearrange("p (kc m) -> p kc m", kc=KC)

        w2_sb = wpool.tile([128, MC * C], BF16, tag="w2", name="w2")
        nc.sync.dma_start(
            w2_sb[:].rearrange("p (mc m) -> p mc m", mc=MC),
            w2_ext[:, :].rearrange("(mc p) m -> p mc m", p=128))
        w2v = w2_sb[:].rearrange("p (mc m) -> p mc m", mc=MC)

        def dgload(ext, tag):
            t = wpool.tile([128, N_TAPS * KC * 128], FP8, tag=tag, name=tag)
            nc.sync.dma_start(
                t[:].rearrange("p (blk d) -> p blk d", d=128),
                ext[:, :].rearrange("(blk p) d -> p blk d", p=128))
            return t[:].rearrange("p (kc t d) -> p kc t d", kc=KC, t=N_TAPS)

        qdv = dgload(qd_ext, "qdall")
        kdv = dgload(kd_ext, "kdall")

        def bload(ext, tag):
            t = wpool.tile([128, MC], F32, tag=tag, name=tag)
            nc.sync.dma_start(
                t[:].unsqueeze(2),
                ext[:, :].rearrange("(m p) o -> p m o", p=128))
            return t

        bq_sb = bload(bq_ext, "bq")
        bk_sb = bload(bk_ext, "bk")
        b2_sb = bload(b2_ext, "b2")

        # ones-masks for denominator matmuls (fp8), jc-paired layout [128,2,128]
        maskA = wpool.tile([128, 256], FP8, tag="maskA", name="maskA")
        maskB = wpool.tile([128, 256], FP8, tag="maskB", name="maskB")
        mAv = maskA[:].rearrange("p (s m) -> p s m", s=2)
        mBv = maskB[:].rearrange("p (s m) -> p s m", s=2)
        nc.gpsimd.memset(maskA[:], 0.0)
        nc.gpsimd.memset(mAv[:, :, 0:64], 1.0)
        nc.gpsimd.memset(maskB[:], 0.0)
        nc.gpsimd.memset(mBv[:, :, 64:128], 1.0)

        # ---- x staging: DMA straight into padded tiles ----
        def xp_load(b01):
            tiles = []
            for kc_ in range(KC):
                xp = xpool.tile([128, 2 * PADN], FP8, tag="xp", name="xp")
                xp4 = xp[:].rearrange("p (b r c) -> p b r c", b=2, c=PADR)
                nc.gpsimd.memset(xp4[:, :, 0:1, :], 0.0)
                nc.gpsimd.memset(xp4[:, :, 33:34, :], 0.0)
                nc.gpsimd.memset(xp4[:, :, 1:33, 0:1], 0.0)
                nc.gpsimd.memset(xp4[:, :, 1:33, 33:34], 0.0)
                for bb in range(2):
                    src = x_ext[2 * b01 + bb, kc_ * 128:(kc_ + 1) * 128, :, :]
                    nc.scalar.dma_start(xp4[:, bb, 1:33, 1:33], src)
                tiles.append(xp4)
            return tiles

        def conv_b(xp4s, b):
            """Depthwise convs for batch b (bb = b % 2) on PE, fp8 DoubleRow.
            Returns (y1pair, y1last, y2pair, y2last) fp8 sbuf tiles."""
            bb = b % 2
            y1p = y1pool.tile([128, 2 * HW], FP8, tag="y1p", name="y1p")
            y1l = y1pool.tile([128, HW], FP8, tag="y1l", name="y1l")
            y2p = y2pool.tile([128, 2 * JK], FP8, tag="y2p", name="y2p")
            y2l = y2pool.tile([128, JK], FP8, tag="y2l", name="y2l")
            for kc_ in range(KC):
                xp4 = xp4s[kc_]
                ps1 = psbig.tile([128, HW], F32, tag="ps", name="ps1")
                for n2 in range(2):
                    for pi, (tA, tB) in enumerate(TAP_PAIRS):
                        nc.tensor.matmul(
                            ps1[:, n2 * 512:(n2 + 1) * 512],
                            qdv[:, kc_, 2 * pi:2 * pi + 2, :],
                            _pair_win(xp4, bb, tA, tB, n2=n2, stride=1),
                            start=(pi == 0), stop=(pi == len(TAP_PAIRS) - 1),
                            perf_mode=DR)
                dst1 = y1p[:, kc_ * HW:(kc_ + 1) * HW] if kc_ < 2 else y1l[:]
                nc.vector.tensor_scalar_mul(dst1, ps1[:], IWS)
                psf2 = pssml.tile([128, 512], F32, tag="pk", name="ps2")
                ps2 = psf2[:, 0:JK]
                for pi, (tA, tB) in enumerate(TAP_PAIRS):
                    nc.tensor.matmul(
                        ps2,
                        kdv[:, kc_, 2 * pi:2 * pi + 2, :],
                        _pair_win(xp4, bb, tA, tB, stride=2),
                        start=(pi == 0), stop=(pi == len(TAP_PAIRS) - 1),
                        perf_mode=DR)
                dst2 = y2p[:, kc_ * JK:(kc_ + 1) * JK] if kc_ < 2 else y2l[:]
                nc.scalar.mul(dst2, ps2, IWS)
            return (y1p[:].rearrange("p (s n) -> p s n", s=2), y1l[:],
                    y2p[:].rearrange("p (s n) -> p s n", s=2), y2l[:])

        def ab_phase(b, y1p, y1l, y2p, y2l):
            """Pointwise projections -> q [3][128,1024], k [3][128,256],
            vT2 [128, 2, 384] (jc-paired), all fp8."""
            q_sb = []
            for mc_ in range(MC):
                qt = qpool.tile([128, HW], FP8, tag="q", name="qsb")
                ps = psbig.tile([128, HW], F32, tag="ps", name="psA")
                for n2 in range(2):
                    sl = slice(n2 * 512, (n2 + 1) * 512)
                    nc.tensor.matmul(
                        ps[:, sl], aqv[:, 0:2, mc_ * 128:(mc_ + 1) * 128],
                        y1p[:, :, sl], start=True, stop=False, perf_mode=DR)
                    nc.tensor.matmul(
                        ps[:, sl], aqv[:, 2, mc_ * 128:(mc_ + 1) * 128],
                        y1l[:, sl], start=False, stop=True)
                nc.vector.tensor_scalar(qt[:], ps[:], IWS, bq_sb[:, mc_:mc_ + 1],
                                        AL.mult, AL.add)
                q_sb.append(qt)

            k_sb = []
            for mc_ in range(MC):
                kt = kpool.tile([128, JK], FP8, tag="k", name="ksb")
                psf = pssml.tile([128, 512], F32, tag="pk", name="psBk")
                ps = psf[:, 0:JK]
                nc.tensor.matmul(
                    ps, akv[:, 0:2, mc_ * 128:(mc_ + 1) * 128],
                    y2p[:], start=True, stop=False, perf_mode=DR)
                nc.tensor.matmul(
                    ps, akv[:, 2, mc_ * 128:(mc_ + 1) * 128],
                    y2l[:], start=False, stop=True)
                nc.vector.tensor_scalar(kt[:], ps, IWS, bk_sb[:, mc_:mc_ + 1],
                                        AL.mult, AL.add)
                k_sb.append(kt)

            vt = vpool.tile([128, 2 * INNER], FP8, tag="v", name="vsb")
            vt2 = vt[:].rearrange("p (s m) -> p s m", s=2)
            av2 = av_sb[:][:, 0:2 * INNER].rearrange("p (s m) -> p s m", s=2)
            for jc in range(2):
                psf = pssml.tile([128, 512], F32, tag="pk", name="psBv")
                ps = psf[:, 0:INNER]
                y2pj = y2p[:, :, jc * 128:(jc + 1) * 128]
                nc.tensor.matmul(ps, y2pj, av2, start=True, stop=False,
                                 perf_mode=DR)
                nc.tensor.matmul(ps, y2l[:, jc * 128:(jc + 1) * 128],
                                 avv[:, 2, :], start=False, stop=True)
                nc.scalar.mul(vt2[:, jc, :], ps, IWS)
            return q_sb, k_sb, vt2

        def dots_phase(b, q_sb, k_sb):
            """dots^T + exp -> e tiles [pair][h01] = [128, 2, 1024] fp8."""
            et = [[None, None] for _ in range(NPAIR)]
            for p in range(NPAIR):
                for h01 in range(2):
                    hs = h01 * 64
                    e = epool.tile([128, 2 * HW], FP8, tag="et", name="et")
                    for jc in range(2):
                        psd = psbig.tile([128, HW], F32, tag="ps", name="psd")
                        for ic in range(2):
                            nc.tensor.matmul(
                                psd[:, ic * 512:(ic + 1) * 512],
                                k_sb[p][hs:hs + 64, jc * 128:(jc + 1) * 128],
                                q_sb[p][hs:hs + 64, ic * 512:(ic + 1) * 512],
                                start=True, stop=True,
                                tile_position=(hs, 0))
                        nc.scalar.activation(
                            e[:, jc * HW:(jc + 1) * HW], psd[:], AF.Exp,
                            scale=SCALE)
                    et[p][h01] = e[:].rearrange("p (jc i) -> p jc i", jc=2)
            return et

        def denav_phase(b, et, vt2):
            """Denominators (mask DR matmuls) + attn@v (DR) + normalize."""
            ots = []
            for p in range(NPAIR):
                psn = psbig.tile([128, HW], F32, tag="ps", name="psn")
                for ic in range(2):
                    sl = slice(ic * 512, (ic + 1) * 512)
                    nc.tensor.matmul(psn[:, sl], mAv, et[p][0][:, :, sl],
                                     start=True, stop=False, perf_mode=DR)
                    nc.tensor.matmul(psn[:, sl], mBv, et[p][1][:, :, sl],
                                     start=False, stop=True, perf_mode=DR)
                rec = rpool.tile([128, HW], F32, tag="recip", name="recip")
                nc.vector.reciprocal_approx_fast(out=rec[:], in_=psn[:])

                pso = psbig.tile([128, HW], F32, tag="ps", name="pso")
                for ic in range(2):
                    # h01=0: DoubleRow at tile position (0,0)
                    nc.tensor.matmul(
                        pso[0:64, ic * 512:(ic + 1) * 512],
                        vt2[:, :, p * 128:p * 128 + 64],
                        et[p][0][:, :, ic * 512:(ic + 1) * 512],
                        start=True, stop=True, perf_mode=DR,
                        tile_position=(0, 0))
                    # h01=1: DR + offset tile_position is invalid ISA -> plain
                    for jc in range(2):
                        nc.tensor.matmul(
                            pso[64:128, ic * 512:(ic + 1) * 512],
                            vt2[:, jc, p * 128 + 64:p * 128 + 128],
                            et[p][1][:, jc, ic * 512:(ic + 1) * 512],
                            start=(jc == 0), stop=(jc == 1),
                            tile_position=(0, 64))
                ot = opool.tile([128, HW], BF16, tag="outT", name="outT")
                nc.vector.tensor_tensor(ot[:], pso[:], rec[:], AL.mult)
                ots.append(ot)
            return ots

        def out_phase(b, ots):
            for mc_ in range(MC):
                fin = fpool.tile([128, HW], F32, tag="fin", name="fin")
                ps = psbig.tile([128, HW], F32, tag="ps", name="psE")
                for n2 in range(2):
                    for p in range(NPAIR):
                        nc.tensor.matmul(
                            ps[:, n2 * 512:(n2 + 1) * 512],
                            w2v[:, p, mc_ * 128:(mc_ + 1) * 128],
                            ots[p][:, n2 * 512:(n2 + 1) * 512],
                            start=(p == 0), stop=(p == NPAIR - 1))
                if b % 2 == 0:
                    nc.scalar.activation(fin[:], ps[:], AF.Identity,
                                         bias=b2_sb[:, mc_:mc_ + 1], scale=1.0)
                else:
                    nc.vector.tensor_scalar(fin[:], ps[:], 1.0,
                                            b2_sb[:, mc_:mc_ + 1],
                                            AL.mult, AL.add)
                nc.sync.dma_start(
                    out_ext[b, mc_ * 128:(mc_ + 1) * 128, :, :],
                    fin[:].rearrange("p (h w) -> p h w", w=W))

        # ---- schedule ----
        xp0 = xp_load(0)
        xp1 = xp_load(1)
        y = [None] * 4
        y[0] = conv_b(xp0, 0)
        y[1] = conv_b(xp0, 1)
        qkv0 = ab_phase(0, *y[0])
        qkv1 = ab_phase(1, *y[1])
        e0 = dots_phase(0, qkv0[0], qkv0[1])
        e1 = dots_phase(1, qkv1[0], qkv1[1])
        y[2] = conv_b(xp1, 2)        # PE work covering exp(0)/exp(1) latency
        y[3] = conv_b(xp1, 3)
        o0 = denav_phase(0, e0, qkv0[2])
        o1 = denav_phase(1, e1, qkv1[2])
        out_phase(0, o0)
        out_phase(1, o1)
        qkv2 = ab_phase(2, *y[2])
        qkv3 = ab_phase(3, *y[3])
        e2 = dots_phase(2, qkv2[0], qkv2[1])
        e3 = dots_phase(3, qkv3[0], qkv3[1])
        o2 = denav_phase(2, e2, qkv2[2])
        o3 = denav_phase(3, e3, qkv3[2])
        out_phase(2, o2)
        out_phase(3, o3)

    nc.compile()
    return nc


_NC_CACHE = None


def _get_nc():
    global _NC_CACHE
    if _NC_CACHE is None:
        _NC_CACHE = build_nc()
    return _NC_CACHE


def _prep_host(inputs):
    """Fold BN into pointwise weights; fold v-bias into final bias."""
    f32 = np.float32
    bf16 = ml_dtypes.bfloat16
    inv_q = (inputs['q_gamma'] / np.sqrt(inputs['q_var'] + EPS)).astype(f32)
    sh_q = (inputs['q_beta'] - inputs['q_mean'] * inv_q).astype(f32)
    A_q = (inputs['q_pw'] * inv_q[None, :]).astype(f32)
    b_q = (inputs['q_pw'].astype(f32) @ sh_q).astype(f32)

    inv_kv = (inputs['kv_gamma'] / np.sqrt(inputs['kv_var'] + EPS)).astype(f32)
    sh_kv = (inputs['kv_beta'] - inputs['kv_mean'] * inv_kv).astype(f32)
    A_kv = (inputs['kv_pw'] * inv_kv[None, :]).astype(f32)
    b_kv = (inputs['kv_pw'].astype(f32) @ sh_kv).astype(f32)
    A_k, A_v = A_kv[:INNER], A_kv[INNER:]
    b_k, b_v = b_kv[:INNER], b_kv[INNER:]

    W2 = inputs['out_w'].astype(f32)
    b2 = (inputs['out_b'].astype(f32) + W2 @ b_v).astype(f32)

    # 10 diag blocks per (kc): tap pairs in TAP_PAIRS order, zero block pads
    def diag_blocks(taps):
        out = np.zeros((N_TAPS * C, 128), f32)
        for pi, (tA, tB) in enumerate(TAP_PAIRS):
            for si, t in enumerate((tA, tB)):
                if t is None:
                    continue
                ti = t[0] * 3 + t[1]
                blk_i = 2 * pi + si
                for kc_ in range(KC):
                    blk = np.diag(taps[kc_ * 128:(kc_ + 1) * 128, ti]) * WS
                    r0 = kc_ * N_TAPS * 128 + blk_i * 128
                    out[r0:r0 + 128, :] = blk
        return out

    qdiag = diag_blocks(inputs['q_dw'].reshape(C, 9).astype(f32))
    kvdiag = diag_blocks(inputs['kv_dw'].reshape(C, 9).astype(f32))

    return {
        'qdiag': qdiag.astype(NP_FP8),
        'kvdiag': kvdiag.astype(NP_FP8),
        'aq': np.ascontiguousarray(A_q.T * WS).astype(NP_FP8),
        'ak': np.ascontiguousarray(A_k.T * WS).astype(NP_FP8),
        'av': np.ascontiguousarray(A_v.T * WS).astype(NP_FP8),
        'w2': np.ascontiguousarray(W2.T).astype(bf16),
        'bq': b_q.reshape(INNER, 1),
        'bk': b_k.reshape(INNER, 1),
        'b2': b2.reshape(C, 1),
    }


def _make_in_maps(inputs):
    wmap = _prep_host(inputs)
    x8 = inputs['x'].astype(NP_FP8)
    in_maps = []
    for c in range(N_CORES):
        m = dict(wmap)
        m['x'] = np.ascontiguousarray(x8[c * B_LOC:(c + 1) * B_LOC])
        in_maps.append(m)
    return in_maps


def kernel(**inputs):
    inputs = {k: np.asarray(v) for k, v in inputs.items()}
    nc = _get_nc()
    in_maps = _make_in_maps(inputs)
    res = run_bass_kernel_spmd(nc, in_maps, core_ids=list(range(N_CORES)))
    shards = [res.results[i]['out'] for i in range(N_CORES)]
    return np.concatenate(shards, axis=0).astype(np.float32)
